# revision 1
# baseline (speedup 1.0000x reference)
"""Bi-directional MinGRU kernel for Trainium2 (8 NeuronCores, SPMD).

Problem: x [4, 4096, 1024]; per direction d in {fwd, bwd}:
    k  = x @ Wz_d + bz_d
    A  = sigmoid(-k)           (= 1 - z, the carry coefficient)
    z  = sigmoid(k)
    gp = x @ Wh_d + bh_d
    g  = max(gp + 0.5, sigmoid(gp))      (== where(gp>=0, gp+0.5, sigmoid(gp)))
    h_t = A_t * h_{t-1} + z_t * g_t      (linear first-order scan over S)
    out = concat(h_fwd, h_bwd) @ W_out + b_out

Sharding: 8 cores = (4 batches) x (2 directions). Each core computes the
full hidden state for one (batch, direction) and its half of the final
2H->H projection; the two partial products per batch are summed on host.

Per-core layout: everything is kept transposed ([channel, seq]) so the
sequential scan runs along the free dimension with channels on partitions,
using the native VectorE tensor_tensor_scan instruction.
"""

import os
import numpy as np
from contextlib import ExitStack

import concourse.bass as bass
import concourse.tile as tile
from concourse import bacc, mybir
from concourse.bass_utils import run_bass_kernel_spmd

P = 128          # partitions
S = 4096         # sequence length
D = 1024         # input dim
H = 1024         # hidden dim
SC = 512         # seq chunk (one PSUM bank of fp32)
NSC = S // SC    # 8 seq chunks
ND = D // P      # 8 contraction tiles for GEMM1
NH = H // P      # 8 hidden tiles
NCORES = 8

F32 = mybir.dt.float32

# matmul input modes:
#   "f32r"   - all matmul inputs float32r (fp32 bytes, 1 cyc/row PE path)
#   "hybrid" - gate GEMMs (x, Wz, Wh) in bf16 (their error is damped by the
#              sigmoids), output GEMM (h, Wo) in float32r
#   "bf16"   - everything bf16
# float32r must be declared end-to-end (walrus birverifier requires the
# producer chain to be f32r-typed); the raw bytes are plain fp32.
# Default bf16: l2-rel 2.7e-3 => resid_var 7.2e-6, 14x under the
# concourse-standard gate (resid_var < 1e-4); fastest measured config.
MM_MODE = os.environ.get("BIMINGRU_MM_MODE", "bf16")

BF16 = mybir.dt.bfloat16
F32R = mybir.dt.float32r
if MM_MODE == "bf16":
    X_DT, O_DT = BF16, BF16
elif MM_MODE == "hybrid":
    X_DT, O_DT = BF16, F32R
elif MM_MODE == "f32r":
    X_DT, O_DT = F32R, F32R
else:
    X_DT, O_DT = F32, F32
H_DT = O_DT                      # scan output dtype (GEMM3 rhs)


def _np_dt(dt):
    if dt == BF16:
        import ml_dtypes
        return np.dtype(ml_dtypes.bfloat16)
    return np.dtype(np.float32)


def _mm(ap):
    return ap


def _build_module():
    nc = bacc.Bacc("TRN2", target_bir_lowering=False, debug=False)

    # All inputs are host-blocked so every SBUF working set is ONE contiguous
    # DMA (the sync engine's ~0.65us per-DMA issue cost dominates the ramp):
    #   xT row j*128+p, col d*512+c   = x^T[d*128+p, j*512+c]   (chunk-blocked)
    #   Wz/Wh row i*128+p, col d*128+c = W[d*128+p, i*128+c]    (i-blocked)
    #   Wo row o*128+p, col i*128+c    = W_half[i*128+p, o*128+c] (o-blocked)
    #   biasT [128, 4*NH] = [bz | -bz | bh | bh+0.5] per-partition columns
    xT = nc.dram_tensor("xT", [D, S], X_DT, kind="ExternalInput").ap()
    Wz = nc.dram_tensor("Wz", [D, H], X_DT, kind="ExternalInput").ap()
    Wh = nc.dram_tensor("Wh", [D, H], X_DT, kind="ExternalInput").ap()
    Wo = nc.dram_tensor("Wo", [H, H], O_DT, kind="ExternalInput").ap()
    biasT = nc.dram_tensor("biasT", [P, 4 * NH], F32, kind="ExternalInput").ap()
    outT = nc.dram_tensor("outT", [H, S], F32, kind="ExternalOutput").ap()

    AF = mybir.ActivationFunctionType
    OP = mybir.AluOpType

    with tile.TileContext(nc) as tc, ExitStack() as ctx:
        wpool = ctx.enter_context(tc.tile_pool(name="w", bufs=1))
        xpool = ctx.enter_context(tc.tile_pool(name="x", bufs=2))
        pspool = ctx.enter_context(tc.tile_pool(name="ps", bufs=2, space="PSUM"))
        ewpool = ctx.enter_context(tc.tile_pool(name="ew", bufs=2))
        hpool = ctx.enter_context(tc.tile_pool(name="h", bufs=2))
        opool = ctx.enter_context(tc.tile_pool(name="o", bufs=3))

        # --- PE warm-up: the first real matmul can't start until ~12us of
        # input DMA lands, and a cold PE then runs at 1.2GHz for another
        # ~3.4us (HAM). Burn that idle window with dummy matmuls on
        # memset-zero tiles so the HAM un-throttles before real work
        # arrives. The dummy PSUM tile reuses the psK tag (no extra bank).
        wdum = ewpool.tile([P, P], X_DT, tag="wdum", name="wdum")
        nc.vector.memset(wdum[:], 0)
        rdum = ewpool.tile([P, SC], X_DT, tag="rdum", name="rdum")
        nc.vector.memset(rdum[:], 0)
        # 10 dummies: they pace at ~0.33us each (WAW-serialized), so this
        # ends ~11.5us — right when HAM warms (8.1+3.4) and the first real
        # operands land; more would push the real stream start back.
        psdum = pspool.tile([P, SC], F32, tag="psK", bufs=3, name="psdum")
        for _ in range(10):
            nc.tensor.matmul(psdum[:], wdum[:], rdum[:], start=True, stop=True)

        x_chunks = {}

        def load_x_chunk(j):
            # one DMA per chunk: [128, ND*SC] with free dim (d, c)
            xt = xpool.tile([P, ND * SC], X_DT, tag="xb", name=f"xb_{j}")
            nc.sync.dma_start(xt[:], xT[j * P:(j + 1) * P, :])
            x_chunks[j] = xt

        # Startup: x chunk 0 is on the critical path to the first matmul, so
        # split it into 4 DMAs (parallel queues + per-MM wait granularity);
        # then the i-blocked Wz/Wh tiles interleaved — K(0,i) unblocks as
        # soon as block WzB[i] lands, so the PE ramps with the DMA stream.
        Wz_t, Wh_t, Wo_t = [], [], []
        xt0 = xpool.tile([P, ND * SC], X_DT, tag="xb", name="xb_0")
        QS = ND * SC // 4
        nc.sync.dma_start(xt0[:, 0:QS], xT[0:P, 0:QS])
        wzt = wpool.tile([P, H], X_DT, tag="wz0", name="wz0")
        nc.sync.dma_start(wzt[:], Wz[0:P, :])
        Wz_t.append(wzt)
        for q in range(1, 4):
            nc.sync.dma_start(xt0[:, q * QS:(q + 1) * QS],
                              xT[0:P, q * QS:(q + 1) * QS])
        x_chunks[0] = xt0

        bias_sb = wpool.tile([P, 4 * NH], F32, tag="bias", name="bias_sb")
        nc.sync.dma_start(bias_sb[:], biasT[:, :])
        bz_sb = bias_sb[:, 0:NH]
        nbz_sb = bias_sb[:, NH:2 * NH]
        bh_sb = bias_sb[:, 2 * NH:3 * NH]
        bh5_sb = bias_sb[:, 3 * NH:4 * NH]

        for i in range(1, NH):
            wzt = wpool.tile([P, H], X_DT, tag=f"wz{i}", name=f"wz{i}")
            nc.sync.dma_start(wzt[:], Wz[i * P:(i + 1) * P, :])
            Wz_t.append(wzt)
            wht = wpool.tile([P, H], X_DT, tag=f"wh{i-1}", name=f"wh{i-1}")
            nc.sync.dma_start(wht[:], Wh[(i - 1) * P:i * P, :])
            Wh_t.append(wht)
        wht = wpool.tile([P, H], X_DT, tag=f"wh{NH-1}", name=f"wh{NH-1}")
        nc.sync.dma_start(wht[:], Wh[(NH - 1) * P:NH * P, :])
        Wh_t.append(wht)

        def load_wo():
            for o in range(NH):
                wot = wpool.tile([P, H], O_DT, tag=f"wo{o}", name=f"wo{o}")
                nc.sync.dma_start(wot[:], Wo[o * P:(o + 1) * P, :])
                Wo_t.append(wot)

        h_tiles = [[None] * NH for _ in range(NSC)]

        stash = {}

        def emit_k(j, i):
            xc = x_chunks[j]
            psK = pspool.tile([P, SC], F32, tag="psK", bufs=3,
                              name=f"psK_{j}_{i}")
            for d in range(ND):
                nc.tensor.matmul(
                    psK[:], _mm(Wz_t[i][:, d * P:(d + 1) * P]),
                    _mm(xc[:, d * SC:(d + 1) * SC]),
                    start=(d == 0), stop=(d == ND - 1))
            A = ewpool.tile([P, SC], F32, tag="A", bufs=3, name=f"A_{j}_{i}")
            nc.scalar.activation(A[:], psK[:], AF.Sigmoid,
                                 bias=nbz_sb[:, i:i + 1], scale=-1.0)
            z = ewpool.tile([P, SC], F32, tag="z", bufs=3, name=f"z_{j}_{i}")
            nc.scalar.activation(z[:], psK[:], AF.Sigmoid,
                                 bias=bz_sb[:, i:i + 1], scale=1.0)
            stash[(j, i)] = (A, z)

        def emit_g(j, i):
            xc = x_chunks[j]
            psG = pspool.tile([P, SC], F32, tag="psG", bufs=3,
                              name=f"psG_{j}_{i}")
            for d in range(ND):
                nc.tensor.matmul(
                    psG[:], _mm(Wh_t[i][:, d * P:(d + 1) * P]),
                    _mm(xc[:, d * SC:(d + 1) * SC]),
                    start=(d == 0), stop=(d == ND - 1))
            A, z = stash.pop((j, i))
            sg = ewpool.tile([P, SC], F32, tag="sg", name=f"sg_{j}_{i}")
            nc.scalar.activation(sg[:], psG[:], AF.Sigmoid,
                                 bias=bh_sb[:, i:i + 1], scale=1.0)
            g = ewpool.tile([P, SC], F32, tag="g", name=f"g_{j}_{i}")
            nc.vector.scalar_tensor_tensor(g[:], psG[:], bh5_sb[:, i:i + 1],
                                           sg[:], op0=OP.add, op1=OP.max)
            Bv = ewpool.tile([P, SC], F32, tag="B", name=f"B_{j}_{i}")
            nc.vector.tensor_tensor(Bv[:], z[:], g[:], op=OP.mult)

            ht = hpool.tile([P, SC], H_DT, tag=f"h{i}", name=f"h_{j}_{i}")
            init = 0.0 if j == 0 else h_tiles[j - 1][i][:, SC - 1:SC]
            nc.vector.tensor_tensor_scan(ht[:], A[:], Bv[:], initial=init,
                                         op0=OP.mult, op1=OP.add)
            h_tiles[j][i] = ht

        def emit_o(j, o):
            psO = pspool.tile([P, SC], F32, tag="psO", name=f"psO_{j}_{o}")
            for i in range(NH):
                nc.tensor.matmul(
                    psO[:], _mm(Wo_t[o][:, i * P:(i + 1) * P]),
                    _mm(h_tiles[j][i][:]),
                    start=(i == 0), stop=(i == NH - 1))
            oc = opool.tile([P, SC], F32, tag="oc", name=f"oc_{j}_{o}")
            nc.scalar.copy(oc[:], psO[:])
            nc.sync.dma_start(outT[o * P:(o + 1) * P, j * SC:(j + 1) * SC], oc[:])

        # Software pipeline. Per chunk j the PE group order is
        #   K0 K1 [G0 O0] [K2 G1 O1] [K3 G2 O2] ... [K7 G6 O6] [G7 O7]
        # where O* are the GEMM3 groups of chunk j-1. Interleaving the O
        # groups keeps ~2 PE groups between G(i) and the DVE/ACT chain that
        # releases its PSUM bank, so the PE never stalls on the elementwise
        # tail. x(j+1) is prefetched at the head of chunk j; Wo loads are
        # issued at the head of chunk 1 (first needed by GEMM3 of chunk 0).
        for j in range(NSC):
            if j + 1 < NSC:
                load_x_chunk(j + 1)
            if j == 1:
                load_wo()
            emit_k(j, 0)
            emit_k(j, 1)
            for i in range(NH):
                if i + 2 < NH:
                    emit_k(j, i + 2)
                emit_g(j, i)
                if j >= 1:
                    emit_o(j - 1, i)
        for o in range(NH - 1):
            emit_o(NSC - 1, o)
        # final O group split into two N=256 halves so the first half's
        # copy+store overlaps the second half's matmuls (shorter serial
        # tail before the drain barrier); PSUM/SBUF tags are reused so no
        # extra banks are allocated
        j, o = NSC - 1, NH - 1
        HC = SC // 2
        for half in range(2):
            psO = pspool.tile([P, HC], F32, tag="psO", name=f"psOt_{half}")
            for i in range(NH):
                nc.tensor.matmul(
                    psO[:], _mm(Wo_t[o][:, i * P:(i + 1) * P]),
                    _mm(h_tiles[j][i][:, half * HC:(half + 1) * HC]),
                    start=(i == 0), stop=(i == NH - 1))
            oc = opool.tile([P, HC], F32, tag="oc", name=f"oct_{half}")
            nc.scalar.copy(oc[:], psO[:])
            nc.sync.dma_start(
                outT[o * P:(o + 1) * P,
                     j * SC + half * HC:j * SC + (half + 1) * HC], oc[:])

    nc.compile()
    return nc


_CACHE = {}


def _get_module():
    if "nc" not in _CACHE:
        _CACHE["nc"] = _build_module()
    return _CACHE["nc"]


def _make_in_maps(x, Wz_f, bz_f, Wh_f, bh_f, Wz_b, bz_b, Wh_b, bh_b, W_out, b_out):
    np_x = _np_dt(X_DT)
    np_o = _np_dt(O_DT)
    f32 = np.float32

    def blk_w(w, dt):
        # [D, H] -> blocked: out[i*128+p, d*128+c] = w[d*128+p, i*128+c]
        w = np.asarray(w, dtype=f32).reshape(ND, P, NH, P)
        return np.ascontiguousarray(
            w.transpose(2, 1, 0, 3).reshape(H, D), dtype=dt)

    def blk_x(xb, rev):
        # [S, D] -> blocked: out[j*128+p, d*512+c] = x[j*512+c, d*128+p]
        if rev:
            xb = xb[::-1]
        xb = xb.reshape(NSC, SC, ND, P)
        return np.ascontiguousarray(
            xb.transpose(0, 3, 2, 1).reshape(NSC * P, ND * SC), dtype=np_x)

    x = np.asarray(x, dtype=f32)
    Wz_fc, Wh_fc = blk_w(Wz_f, np_x), blk_w(Wh_f, np_x)
    Wz_bc, Wh_bc = blk_w(Wz_b, np_x), blk_w(Wh_b, np_x)
    W_out = np.asarray(W_out)
    Wo_fc = blk_w(W_out[:H], np_o)      # fwd half rows of W_out
    Wo_bc = blk_w(W_out[H:], np_o)      # bwd half rows

    def bias_pack(b_z, b_h):
        def col(v):  # [H] -> [128, NH] with col i = h-tile i
            return np.asarray(v, dtype=f32).reshape(NH, P).T
        b_z = np.asarray(b_z, dtype=f32)
        b_h = np.asarray(b_h, dtype=f32)
        return {"biasT": np.ascontiguousarray(np.concatenate(
            [col(b_z), col(-b_z), col(b_h), col(b_h + 0.5)], axis=1))}

    bias_f = bias_pack(bz_f, bh_f)
    bias_b = bias_pack(bz_b, bh_b)

    in_maps = []
    for b in range(4):
        xT_f = blk_x(x[b], rev=False)
        xT_b = blk_x(x[b], rev=True)
        in_maps.append({"xT": xT_f, "Wz": Wz_fc, "Wh": Wh_fc, "Wo": Wo_fc,
                        **bias_f})
        in_maps.append({"xT": xT_b, "Wz": Wz_bc, "Wh": Wh_bc, "Wo": Wo_bc,
                        **bias_b})
    return in_maps


def _assemble(results, b_out):
    out = np.empty((4, S, H), np.float32)
    for b in range(4):
        out[b] = results[2 * b]["outT"].T
        out[b] += results[2 * b + 1]["outT"].T
    out += np.asarray(b_out, dtype=np.float32)
    return out


def kernel(x, Wz_f, bz_f, Wh_f, bh_f, Wz_b, bz_b, Wh_b, bh_b, W_out, b_out):
    nc = _get_module()
    in_maps = _make_in_maps(x, Wz_f, bz_f, Wh_f, bh_f,
                            Wz_b, bz_b, Wh_b, bh_b, W_out, b_out)
    res = run_bass_kernel_spmd(nc, in_maps, core_ids=list(range(NCORES)))
    return _assemble(res.results, b_out)



# revision 4
# speedup vs baseline: 1.2405x; 1.2405x over previous
"""Bi-directional MinGRU kernel for Trainium2 (8 NeuronCores, SPMD).

Problem: x [4, 4096, 1024]; per direction d in {fwd, bwd}:
    k  = x @ Wz_d + bz_d
    A  = sigmoid(-k)           (= 1 - z, the carry coefficient)
    z  = sigmoid(k)
    gp = x @ Wh_d + bh_d
    g  = max(gp + 0.5, sigmoid(gp))      (== where(gp>=0, gp+0.5, sigmoid(gp)))
    h_t = A_t * h_{t-1} + z_t * g_t      (linear first-order scan over S)
    out = concat(h_fwd, h_bwd) @ W_out + b_out

Sharding: 8 cores = (4 batches) x (2 directions). Each core computes the
full hidden state for one (batch, direction) and its half of the final
2H->H projection; the two partial products per batch are summed on host.

Per-core layout: everything is kept transposed ([channel, seq]) so the
sequential scan runs along the free dimension with channels on partitions,
using the native VectorE tensor_tensor_scan instruction.

Precision: the K GEMM (gate pre-activation) runs fully in fp8-e4m3 with
DoubleRow matmuls (2 k-tiles contracted per PE pass, ~1.7x the bf16 rate);
the G GEMM runs its first FG8 k-tiles in fp8-DR and the rest in bf16
(mixed accumulation in one PSUM group, all scales 1); the output GEMM and
h storage stay bf16. Sigmoid damping keeps the e4m3 noise acceptable:
simulated end-to-end max-rel error 1.6e-2 vs the 2e-2 gate (bf16-only is
3.3e-3). Elementwise tiles (A/z/sg/g/B) are bf16 for 2x DVE throughput.
"""

import os
import numpy as np
from contextlib import ExitStack

import concourse.bass as bass
import concourse.tile as tile
from concourse import bacc, mybir
from concourse.bass_utils import run_bass_kernel_spmd

P = 128          # partitions
S = 4096         # sequence length
D = 1024         # input dim
H = 1024         # hidden dim
SC = 512         # seq chunk (one PSUM bank of fp32)
NSC = S // SC    # 8 seq chunks
ND = D // P      # 8 contraction tiles for the input GEMMs
NH = H // P      # 8 hidden tiles
NCORES = 8

# FG8: number of k-tiles (of 8) of the G GEMM computed in fp8-DoubleRow.
# 2 is the sim-validated maximum with margin (rel 1.6e-2 < 2e-2 gate);
# 0 falls back to all-bf16 G.
FG8 = int(os.environ.get("BIMINGRU_FG8", "2"))
assert FG8 % 2 == 0
NG16 = ND - FG8  # bf16 k-tiles in G
DUMN = int(os.environ.get("BIMINGRU_DUMN", "7"))

F32 = mybir.dt.float32
BF16 = mybir.dt.bfloat16
F8 = mybir.dt.float8e4
DR = mybir.MatmulPerfMode.DoubleRow

X_DT = BF16      # G-path x / Wh / Wo dtype
O_DT = BF16
H_DT = BF16      # scan output dtype (GEMM3 rhs)
EW_DT = BF16     # elementwise tiles (A, z, sg, g, B)


def _np_dt(dt):
    import ml_dtypes
    if dt == BF16:
        return np.dtype(ml_dtypes.bfloat16)
    if dt == F8:
        return np.dtype(ml_dtypes.float8_e4m3)
    return np.dtype(np.float32)


def _build_module():
    nc = bacc.Bacc("TRN2", target_bir_lowering=False, debug=False)

    # All inputs are host-blocked so every SBUF working set is ONE contiguous
    # DMA (the ~0.65us per-DMA issue cost dominates the ramp):
    #   xT8 [j*128+p, d, c]   = x[j*512+c, d*128+p]     (fp8, all 8 d-tiles)
    #   xT16 [j*128+p, dd, c] = x[j*512+c, (FG8+dd)*128+p]  (bf16 d-tiles)
    #   Wz8 [i*128+p, d, c]   = Wz[d*128+p, i*128+c]    (fp8)
    #   Wh8 [i*128+p, d, c]   = Wh[d*128+p, i*128+c], d < FG8
    #   Wh16 [i*128+p, dd*128+c] = Wh[(FG8+dd)*128+p, i*128+c]
    #   Wo [o*128+p, i*128+c] = W_half[i*128+p, o*128+c]
    #   biasT [128, 4*NH] = [bz | -bz | bh | bh+0.5] per-partition columns
    xT8 = nc.dram_tensor("xT8", [NSC * P, ND, SC], F8, kind="ExternalInput").ap()
    xT16 = nc.dram_tensor("xT16", [NSC * P, NG16, SC], X_DT,
                          kind="ExternalInput").ap()
    Wz8 = nc.dram_tensor("Wz8", [H, ND, P], F8, kind="ExternalInput").ap()
    Wh8 = nc.dram_tensor("Wh8", [H, FG8, P], F8, kind="ExternalInput").ap()
    Wh16 = nc.dram_tensor("Wh16", [H, NG16 * P], X_DT,
                          kind="ExternalInput").ap()
    Wo = nc.dram_tensor("Wo", [H, H], O_DT, kind="ExternalInput").ap()
    biasT = nc.dram_tensor("biasT", [P, 4 * NH], F32, kind="ExternalInput").ap()
    outT = nc.dram_tensor("outT", [H, S], F32, kind="ExternalOutput").ap()

    AF = mybir.ActivationFunctionType
    OP = mybir.AluOpType

    with tile.TileContext(nc) as tc, ExitStack() as ctx:
        wpool = ctx.enter_context(tc.tile_pool(name="w", bufs=1))
        xpool = ctx.enter_context(tc.tile_pool(name="x", bufs=2))
        pspool = ctx.enter_context(tc.tile_pool(name="ps", bufs=2, space="PSUM"))
        ewpool = ctx.enter_context(tc.tile_pool(name="ew", bufs=2))
        hpool = ctx.enter_context(tc.tile_pool(name="h", bufs=2))
        opool = ctx.enter_context(tc.tile_pool(name="o", bufs=3))

        # --- PE warm-up: the first real matmul can't start until the first
        # weight+x DMAs land (~10-11us incl. the ~7us framework preamble),
        # and a cold PE runs at 1.2GHz until it has been busy ~3.4us (HAM).
        # Burn the DMA-fill window with dummy matmuls on memset-zero tiles
        # so the HAM un-throttles right when real work arrives.
        wdum = ewpool.tile([P, P], X_DT, tag="wdum", name="wdum")
        nc.vector.memset(wdum[:], 0)
        rdum = ewpool.tile([P, SC], X_DT, tag="rdum", name="rdum")
        nc.vector.memset(rdum[:], 0)
        psdum = pspool.tile([P, SC], F32, tag="psK", bufs=3, name="psdum")
        for _ in range(DUMN):
            nc.tensor.matmul(psdum[:], wdum[:], rdum[:], start=True, stop=True)

        x8_chunks = {}
        x16_chunks = {}

        def load_x_chunk(j):
            x8 = xpool.tile([P, ND, SC], F8, tag="x8", name=f"x8_{j}")
            nc.sync.dma_start(x8[:], xT8[j * P:(j + 1) * P])
            x8_chunks[j] = x8
            x16 = xpool.tile([P, NG16, SC], X_DT, tag="x16", name=f"x16_{j}")
            nc.sync.dma_start(x16[:], xT16[j * P:(j + 1) * P])
            x16_chunks[j] = x16

        # Startup: the critical-path DMAs (Wz8[0] + the four DR-pair quarters
        # of x8 chunk 0) are issued in parallel from four engines -- each
        # engine's dynamic DMA queue is distinct, so both the ~0.65us issue
        # cost and the HBM streams parallelize. Everything else follows:
        # sync carries the Wz8 stream (K(0,i) unblocks as Wz8[i] lands),
        # scalar the small Wh8 tiles, vector/gpsimd the Wh16 tiles.
        Wz8_t, Wh8_t, Wh16_t, Wo_t = [], [], [], []
        wzt = wpool.tile([P, ND, P], F8, tag="wz0", name="wz0")
        nc.sync.dma_start(wzt[:], Wz8[0:P])
        Wz8_t.append(wzt)

        xt8_0 = xpool.tile([P, ND, SC], F8, tag="x8", name="x8_0")
        for t, eng in ((0, nc.scalar), (1, nc.gpsimd), (2, nc.sync),
                       (3, nc.sync)):
            eng.dma_start(xt8_0[:, 2 * t:2 * t + 2, :],
                          xT8[0:P, 2 * t:2 * t + 2, :])
        x8_chunks[0] = xt8_0

        bias_sb = wpool.tile([P, 4 * NH], F32, tag="bias", name="bias_sb")
        nc.scalar.dma_start(bias_sb[:], biasT[:, :])
        bz_sb = bias_sb[:, 0:NH]
        nbz_sb = bias_sb[:, NH:2 * NH]
        bh_sb = bias_sb[:, 2 * NH:3 * NH]
        bh5_sb = bias_sb[:, 3 * NH:4 * NH]

        xt16_0 = xpool.tile([P, NG16, SC], X_DT, tag="x16", name="x16_0")
        nc.gpsimd.dma_start(xt16_0[:], xT16[0:P])
        x16_chunks[0] = xt16_0

        for i in range(1, NH):
            wzt = wpool.tile([P, ND, P], F8, tag=f"wz{i}", name=f"wz{i}")
            nc.sync.dma_start(wzt[:], Wz8[i * P:(i + 1) * P])
            Wz8_t.append(wzt)
        for i in range(NH):
            # keep scalar's issue chain short: its first real op (A of
            # chunk 0) is due ~12us in
            eng = nc.scalar if i == 0 else nc.gpsimd
            wht = wpool.tile([P, FG8, P], F8, tag=f"wh8_{i}", name=f"wh8_{i}")
            eng.dma_start(wht[:], Wh8[i * P:(i + 1) * P])
            Wh8_t.append(wht)
        for i in range(NH):
            eng = nc.scalar if i == 0 else nc.gpsimd
            wht = wpool.tile([P, NG16 * P], X_DT, tag=f"wh16_{i}",
                             name=f"wh16_{i}")
            eng.dma_start(wht[:], Wh16[i * P:(i + 1) * P])
            Wh16_t.append(wht)

        def load_wo():
            for o in range(NH):
                wot = wpool.tile([P, H], O_DT, tag=f"wo{o}", name=f"wo{o}")
                nc.sync.dma_start(wot[:], Wo[o * P:(o + 1) * P, :])
                Wo_t.append(wot)

        h_tiles = [[None] * NH for _ in range(NSC)]

        stash = {}

        def emit_k(j, i):
            xc = x8_chunks[j]
            psK = pspool.tile([P, SC], F32, tag="psK", bufs=3,
                              name=f"psK_{j}_{i}")
            for t in range(ND // 2):
                nc.tensor.matmul(
                    psK[:], Wz8_t[i][:, 2 * t:2 * t + 2, :],
                    xc[:, 2 * t:2 * t + 2, :],
                    start=(t == 0), stop=(t == ND // 2 - 1), perf_mode=DR)
            A = ewpool.tile([P, SC], EW_DT, tag="A", bufs=3, name=f"A_{j}_{i}")
            nc.scalar.activation(A[:], psK[:], AF.Sigmoid,
                                 bias=nbz_sb[:, i:i + 1], scale=-1.0)
            z = ewpool.tile([P, SC], EW_DT, tag="z", bufs=3, name=f"z_{j}_{i}")
            nc.scalar.activation(z[:], psK[:], AF.Sigmoid,
                                 bias=bz_sb[:, i:i + 1], scale=1.0)
            stash[(j, i)] = (A, z)

        def emit_g(j, i):
            x8c = x8_chunks[j]
            x16c = x16_chunks[j]
            psG = pspool.tile([P, SC], F32, tag="psG", bufs=3,
                              name=f"psG_{j}_{i}")
            for t in range(FG8 // 2):
                nc.tensor.matmul(
                    psG[:], Wh8_t[i][:, 2 * t:2 * t + 2, :],
                    x8c[:, 2 * t:2 * t + 2, :],
                    start=(t == 0), stop=False, perf_mode=DR)
            for dd in range(NG16):
                nc.tensor.matmul(
                    psG[:], Wh16_t[i][:, dd * P:(dd + 1) * P],
                    x16c[:, dd, :],
                    start=(FG8 == 0 and dd == 0), stop=(dd == NG16 - 1))
            A, z = stash.pop((j, i))
            sg = ewpool.tile([P, SC], EW_DT, tag="sg", name=f"sg_{j}_{i}")
            nc.scalar.activation(sg[:], psG[:], AF.Sigmoid,
                                 bias=bh_sb[:, i:i + 1], scale=1.0)
            g = ewpool.tile([P, SC], EW_DT, tag="g", name=f"g_{j}_{i}")
            nc.vector.scalar_tensor_tensor(g[:], psG[:], bh5_sb[:, i:i + 1],
                                           sg[:], op0=OP.add, op1=OP.max)
            Bv = ewpool.tile([P, SC], EW_DT, tag="B", name=f"B_{j}_{i}")
            nc.vector.tensor_tensor(Bv[:], z[:], g[:], op=OP.mult)

            ht = hpool.tile([P, SC], H_DT, tag=f"h{i}", name=f"h_{j}_{i}")
            init = 0.0 if j == 0 else h_tiles[j - 1][i][:, SC - 1:SC]
            nc.vector.tensor_tensor_scan(ht[:], A[:], Bv[:], initial=init,
                                         op0=OP.mult, op1=OP.add)
            h_tiles[j][i] = ht

        def emit_o(j, o):
            psO = pspool.tile([P, SC], F32, tag="psO", name=f"psO_{j}_{o}")
            for i in range(NH):
                nc.tensor.matmul(
                    psO[:], Wo_t[o][:, i * P:(i + 1) * P],
                    h_tiles[j][i][:],
                    start=(i == 0), stop=(i == NH - 1))
            oc = opool.tile([P, SC], F32, tag="oc", name=f"oc_{j}_{o}")
            nc.scalar.copy(oc[:], psO[:])
            nc.sync.dma_start(outT[o * P:(o + 1) * P, j * SC:(j + 1) * SC], oc[:])

        # Software pipeline. Per chunk j the PE group order is
        #   K0 K1 [G0 O0] [K2 G1 O1] [K3 G2 O2] ... [K7 G6 O6] [G7 O7]
        # where O* are the GEMM3 groups of chunk j-1. Interleaving the O
        # groups keeps ~2 PE groups between G(i) and the DVE/ACT chain that
        # releases its PSUM bank, so the PE never stalls on the elementwise
        # tail. x(j+1) is prefetched at the head of chunk j; Wo loads are
        # issued at the head of chunk 1 (first needed by GEMM3 of chunk 0).
        # The LAST chunk instead runs all its K/G groups first and defers
        # the O groups of chunk NSC-2 to the end: the final scan then
        # completes while the PE chews through those, so the closing
        # O(NSC-1, *) series starts with no scan-wait gap.
        for j in range(NSC):
            last = j == NSC - 1
            if not last:
                load_x_chunk(j + 1)
            if j == 1:
                load_wo()
            emit_k(j, 0)
            emit_k(j, 1)
            for i in range(NH):
                if i + 2 < NH:
                    emit_k(j, i + 2)
                emit_g(j, i)
                if j >= 1 and not last:
                    emit_o(j - 1, i)
        for i in range(NH):
            emit_o(NSC - 2, i)
        for o in range(NH - 1):
            emit_o(NSC - 1, o)
        # final O group split into two N=256 halves so the first half's
        # copy+store overlaps the second half's matmuls (shorter serial
        # tail before the drain barrier); PSUM/SBUF tags are reused so no
        # extra banks are allocated
        j, o = NSC - 1, NH - 1
        HC = SC // 2
        for half in range(2):
            psO = pspool.tile([P, HC], F32, tag="psO", name=f"psOt_{half}")
            for i in range(NH):
                nc.tensor.matmul(
                    psO[:], Wo_t[o][:, i * P:(i + 1) * P],
                    h_tiles[j][i][:, half * HC:(half + 1) * HC],
                    start=(i == 0), stop=(i == NH - 1))
            oc = opool.tile([P, HC], F32, tag="oc", name=f"oct_{half}")
            nc.scalar.copy(oc[:], psO[:])
            nc.sync.dma_start(
                outT[o * P:(o + 1) * P,
                     j * SC + half * HC:j * SC + (half + 1) * HC], oc[:])

    nc.compile()
    return nc


_CACHE = {}


def _get_module():
    if "nc" not in _CACHE:
        _CACHE["nc"] = _build_module()
    return _CACHE["nc"]


def _make_in_maps(x, Wz_f, bz_f, Wh_f, bh_f, Wz_b, bz_b, Wh_b, bh_b, W_out, b_out):
    np_x = _np_dt(X_DT)
    np_o = _np_dt(O_DT)
    np_8 = _np_dt(F8)
    f32 = np.float32

    def blk_w(w, dt):
        # [D, H] -> blocked: out[i*128+p, d*128+c] = w[d*128+p, i*128+c]
        w = np.asarray(w, dtype=f32).reshape(ND, P, NH, P)
        return np.ascontiguousarray(
            w.transpose(2, 1, 0, 3).reshape(H, D), dtype=dt)

    def blk_x(xb, rev):
        # [S, D] -> blocked: out[j*128+p, d, c] = x[j*512+c, d*128+p]
        if rev:
            xb = xb[::-1]
        xb = xb.reshape(NSC, SC, ND, P)
        return np.ascontiguousarray(xb.transpose(0, 3, 2, 1), dtype=f32)

    x = np.asarray(x, dtype=f32)
    W_out = np.asarray(W_out)

    def w_maps(Wz, Wh):
        wzb = blk_w(Wz, f32)
        whb = blk_w(Wh, f32)
        return {
            "Wz8": np.ascontiguousarray(wzb.reshape(H, ND, P), dtype=np_8),
            "Wh8": np.ascontiguousarray(
                whb.reshape(H, ND, P)[:, :FG8], dtype=np_8),
            "Wh16": np.ascontiguousarray(whb[:, FG8 * P:], dtype=np_x),
        }

    w_f = w_maps(Wz_f, Wh_f)
    w_b = w_maps(Wz_b, Wh_b)
    Wo_fc = blk_w(W_out[:H], np_o)      # fwd half rows of W_out
    Wo_bc = blk_w(W_out[H:], np_o)      # bwd half rows

    def bias_pack(b_z, b_h):
        def col(v):  # [H] -> [128, NH] with col i = h-tile i
            return np.asarray(v, dtype=f32).reshape(NH, P).T
        b_z = np.asarray(b_z, dtype=f32)
        b_h = np.asarray(b_h, dtype=f32)
        return np.ascontiguousarray(np.concatenate(
            [col(b_z), col(-b_z), col(b_h), col(b_h + 0.5)], axis=1))

    bias_f = bias_pack(bz_f, bh_f)
    bias_b = bias_pack(bz_b, bh_b)

    in_maps = []
    for b in range(4):
        for rev, wm, wo, bm in ((False, w_f, Wo_fc, bias_f),
                                (True, w_b, Wo_bc, bias_b)):
            xb = blk_x(x[b], rev)  # [NSC, P, ND, SC] f32
            in_maps.append({
                "xT8": np.ascontiguousarray(
                    xb.reshape(NSC * P, ND, SC), dtype=np_8),
                "xT16": np.ascontiguousarray(
                    xb[:, :, FG8:].reshape(NSC * P, NG16, SC), dtype=np_x),
                "Wo": wo, "biasT": bm, **wm})
    return in_maps


def _assemble(results, b_out):
    out = np.empty((4, S, H), np.float32)
    for b in range(4):
        out[b] = results[2 * b]["outT"].T
        out[b] += results[2 * b + 1]["outT"].T
    out += np.asarray(b_out, dtype=np.float32)
    return out


def kernel(x, Wz_f, bz_f, Wh_f, bh_f, Wz_b, bz_b, Wh_b, bh_b, W_out, b_out):
    nc = _get_module()
    in_maps = _make_in_maps(x, Wz_f, bz_f, Wh_f, bh_f,
                            Wz_b, bz_b, Wh_b, bh_b, W_out, b_out)
    res = run_bass_kernel_spmd(nc, in_maps, core_ids=list(range(NCORES)))
    return _assemble(res.results, b_out)
